# revision 16
# baseline (speedup 1.0000x reference)
"""Trainium2 Bass kernel for nn_ComplexAttentionBlock.

Sharding: data-parallel over batch B=8, one sample per NeuronCore.
Per-core layout strategy:
  - activations kept in transposed ("T") layout [feature, seq] so every
    matmul contracts over the partition dim with zero activation
    transposes after the initial c -> cT transpose.
  - LayerNorm row stats (over features = partitions in T layout) via
    ones-vector matmuls on the PE; broadcast back via gpsimd
    partition_broadcast.
  - attention scores computed directly transposed [kpos, q] so the
    exp() output is directly the lhsT of the attn@v matmul.
  - rowsum + the three heads (score/conf/halt) folded into a single
    [kpos, 4] augmented-V matmul: col0 = ones (rowsum), cols 1..3 =
    V projected through filt*W_head.
  - softmax skips max-subtraction (scores max ~8.5, exp is safe).
  - mm_mode "b2": every heavy matmul operand is split into a bf16
    hi/lo pair; each product becomes 3 bf16 matmuls (hi*hi + hi*lo +
    lo*hi) accumulating in the same PSUM group.  3 cyc/row vs fp32's
    4 cyc/row, with ~2^-16 operand precision.
"""

import os
import numpy as np
from contextlib import ExitStack

import concourse.bass as bass
import concourse.bacc as bacc
import concourse.mybir as mybir
import concourse.tile as tile
from concourse.bass_utils import run_bass_kernel_spmd
from concourse.masks import make_identity

f32 = mybir.dt.float32
f32r = mybir.dt.float32r
bf16 = mybir.dt.bfloat16
AF = mybir.ActivationFunctionType
ALU = mybir.AluOpType

P = 128
D = 512
DT = D // P  # 4 feature tiles
EPS = 1e-6
N_CORES = 8

W_NAMES = ("Wlin_r", "Wlin_i", "Wq_r", "Wq_i", "Wk_r", "Wk_i", "Wv_r", "Wv_i")
VEC_NAMES = ("ln_scale", "ln_shift", "act_bias", "gate_mask")
HEAD_W = ("W_score", "W_conf", "W_halt")
HEAD_B = ("b_score", "b_conf", "b_halt")


def build_program(S=2048, mm_mode="b2", trivial_ln=True, trivial_act=True):
    SC = 512
    NSC = S // SC          # s-chunks for phases A/B
    ST = S // P            # s-tiles (also kpos tiles)
    QC = 256
    NQC = S // QC          # q-chunks for attention
    KT = ST
    assert S % SC == 0 and S % QC == 0
    b2 = (mm_mode == "b2")
    mdt = f32r if mm_mode == "f32r" else f32

    nc = bacc.Bacc("TRN2", target_bir_lowering=False, debug=False,
                   num_devices=N_CORES)

    dram = {}
    for nm in ("raw_real", "raw_imag", "bs_real", "bs_imag"):
        dram[nm] = nc.dram_tensor(nm, [S, D], f32, kind="ExternalInput").ap()
    for nm in W_NAMES:
        dram[nm] = nc.dram_tensor(nm, [D, D], f32, kind="ExternalInput").ap()
    for nm in VEC_NAMES:
        dram[nm] = nc.dram_tensor(nm, [D], f32, kind="ExternalInput").ap()
    for nm in HEAD_W:
        dram[nm] = nc.dram_tensor(nm, [1, 2 * D], f32, kind="ExternalInput").ap()
    for nm in HEAD_B:
        dram[nm] = nc.dram_tensor(nm, [1], f32, kind="ExternalInput").ap()

    out_pr = nc.dram_tensor("pr", [S, D], f32, kind="ExternalOutput").ap()
    out_pi = nc.dram_tensor("pi", [S, D], f32, kind="ExternalOutput").ap()
    out_sc = nc.dram_tensor("score", [S, 1], f32, kind="ExternalOutput").ap()
    out_cf = nc.dram_tensor("confidence", [S, 1], f32, kind="ExternalOutput").ap()
    out_hl = nc.dram_tensor("halt", [S, 1], f32, kind="ExternalOutput").ap()
    out_ft = nc.dram_tensor("filt", [D], f32, kind="ExternalOutput").ap()

    with tile.TileContext(nc) as tc, ExitStack() as top:
        const = top.enter_context(tc.tile_pool(name="const", bufs=1))
        zv = top.enter_context(tc.tile_pool(name="zv", bufs=35))
        dsp = top.enter_context(tc.tile_pool(name="dspill", bufs=1, space="DRAM"))
        zv_bufs = 36 if b2 else 35

        # ---------- pair helpers ----------
        def pair_from(pool, shape, tag, bufs, name, src, sign=1.0):
            """Copy src (f32 PSUM/SBUF AP) into a matmul operand: a bf16
            hi/lo pair in b2 mode, else a single mdt tile."""
            if not b2:
                t = pool.tile(shape, mdt, tag=tag, bufs=bufs, name=name)
                if sign == 1.0:
                    nc.scalar.copy(out=t, in_=src)
                else:
                    nc.scalar.mul(out=t, in_=src, mul=float(sign))
                return t
            hi = pool.tile(shape, bf16, tag=tag + "h", bufs=bufs,
                           name=name + "_h")
            if sign == 1.0:
                nc.scalar.copy(out=hi, in_=src)
            else:
                nc.scalar.mul(out=hi, in_=src, mul=float(sign))
            lo = pool.tile(shape, bf16, tag=tag + "l", bufs=bufs,
                           name=name + "_l")
            # lo = sign*src - hi:  (hi * -1) +/- src
            nc.vector.scalar_tensor_tensor(
                out=lo, in0=hi, scalar=-1.0, in1=src, op0=ALU.mult,
                op1=ALU.add if sign > 0 else ALU.subtract)
            return (hi, lo)

        def psl(x, sl):
            if b2:
                return (x[0][:, sl], x[1][:, sl])
            return x[:, sl]

        def opr(x):
            """matmul-ready view of a single (non-pair) operand."""
            return x.bitcast(f32r) if mm_mode == "f32r" else x

        def emit_group(ps, terms):
            mml = []
            for l, r in terms:
                if b2:
                    mml += [(l[0], r[0]), (l[0], r[1]), (l[1], r[0])]
                else:
                    mml.append((opr(l), opr(r)))
            n = len(mml)
            for i, (l, r) in enumerate(mml):
                nc.tensor.matmul(ps, l, r, start=(i == 0), stop=(i == n - 1))

        qsp = {}
        qkeys = [("r", "h"), ("r", "l"), ("i", "h"), ("i", "l")] if b2 \
            else [("r", ""), ("i", "")]
        for c, h in qkeys:
            qsp[(c, h)] = dsp.tile([D, S], bf16 if b2 else mdt,
                                   tag=f"qsp_{c}{h}", name=f"qsp_{c}{h}")

        ident = const.tile([P, P], f32, tag="ident", name="ident")
        make_identity(nc, ident)
        ones_col = const.tile([P, 1], f32, tag="ones_col", name="ones_col")
        nc.vector.memset(ones_col, 1.0)
        eps_row = const.tile([1, 1], f32, tag="eps_row", name="eps_row")
        nc.vector.memset(eps_row, EPS)

        # gate -> filt, in both [P, DT] (per-partition) and [1, D] (row) forms
        gate_pd = const.tile([P, DT], f32, tag="gate_pd", name="gate_pd")
        nc.sync.dma_start(out=gate_pd,
                          in_=dram["gate_mask"].rearrange("(t p) -> p t", p=P))
        filt_pd = const.tile([P, DT], f32, tag="filt_pd", name="filt_pd")
        nc.scalar.activation(out=filt_pd, in_=gate_pd, func=AF.Sigmoid)
        gate_row = const.tile([1, D], f32, tag="gate_row", name="gate_row")
        nc.sync.dma_start(out=gate_row,
                          in_=dram["gate_mask"].rearrange("(o d) -> o d", o=1))
        filt_row = gate_row
        nc.scalar.activation(out=filt_row, in_=gate_row, func=AF.Sigmoid)
        nc.sync.dma_start(out=out_ft.rearrange("(o d) -> o d", o=1), in_=filt_row)
        filt_bc = const.tile([P, D], f32, tag="filt_bc", name="filt_bc")
        nc.gpsimd.partition_broadcast(filt_bc, filt_row)

        def load_scalar_bc(nm):
            r = const.tile([1, 1], f32, tag=nm + "_r", name=nm + "_r")
            nc.sync.dma_start(out=r, in_=dram[nm].rearrange("(o d) -> o d", o=1))
            t = const.tile([P, 1], f32, tag=nm + "_bc", name=nm + "_bc")
            nc.gpsimd.partition_broadcast(t, r)
            return t

        b_score_bc = load_scalar_bc("b_score")
        b_conf_bc = load_scalar_bc("b_conf")
        b_halt_bc = load_scalar_bc("b_halt")

        if not trivial_ln:
            g_pd = const.tile([P, DT], f32, tag="g_pd", name="g_pd")
            nc.sync.dma_start(out=g_pd,
                              in_=dram["ln_scale"].rearrange("(t p) -> p t", p=P))
            sh_pd = const.tile([P, DT], f32, tag="sh_pd", name="sh_pd")
            nc.sync.dma_start(out=sh_pd,
                              in_=dram["ln_shift"].rearrange("(t p) -> p t", p=P))
        if not trivial_act:
            ab_pd = const.tile([P, DT], f32, tag="ab_pd", name="ab_pd")
            nc.sync.dma_start(out=ab_pd,
                              in_=dram["act_bias"].rearrange("(t p) -> p t", p=P))
            abe_pd = const.tile([P, DT], f32, tag="abe_pd", name="abe_pd")
            nc.vector.tensor_scalar_add(out=abe_pd, in0=ab_pd, scalar1=EPS)

        def load_weightT(wpool, stg, pspool, dram_ap, scales, nm, ld_bufs=6):
            """Load W [D, D] (torch layout [out, in]) and produce transposed
            operand tiles WT[di_tile] scaled by each s in scales."""
            nat = []
            for t in range(DT):
                wn = stg.tile([P, D], f32, tag="ld", bufs=ld_bufs,
                              name=f"{nm}_nat{t}")
                nc.sync.dma_start(out=wn, in_=dram_ap[t * P:(t + 1) * P, :])
                nat.append(wn)
            outs = {s: [] for s in scales}
            for ti in range(DT):
                ps = pspool.tile([P, D], f32, tag="ps_w", bufs=2,
                                 name=f"{nm}_ps{ti}")
                for to in range(DT):
                    nc.tensor.matmul(ps[:, to * P:(to + 1) * P],
                                     nat[to][:, ti * P:(ti + 1) * P], ident,
                                     is_transpose=True, start=True, stop=True,
                                     skip_group_check=True)
                for s in scales:
                    bufs = 8 if s > 0 else 4
                    if b2 and abs(s) != 1.0:
                        tmp = stg.tile([P, D], f32, tag="cn", bufs=8,
                                       name=f"{nm}_tmp_{s}_{ti}")
                        nc.scalar.mul(out=tmp, in_=ps, mul=float(s))
                        wt = pair_from(wpool, [P, D], f"w{s}", bufs,
                                       f"{nm}T_{s}_{ti}", tmp, sign=1.0)
                    else:
                        wt = pair_from(wpool, [P, D], f"w{s}", bufs,
                                       f"{nm}T_{s}_{ti}", ps, sign=s)
                    outs[s].append(wt)
            return outs, nat

        z2r = [[None] * DT for _ in range(NSC)]
        z2i = [[None] * DT for _ in range(NSC)]

        # ------------------------------------------------------------------
        # Phase A: combine -> transpose -> ComplexLinear -> LN+ModReLU -> z2T
        # ------------------------------------------------------------------
        with ExitStack() as phA:
            wl = phA.enter_context(tc.tile_pool(name="wlin", bufs=1))
            stg = phA.enter_context(tc.tile_pool(name="stgA", bufs=8))
            cpool = phA.enter_context(tc.tile_pool(name="cT", bufs=8))
            lnp = phA.enter_context(tc.tile_pool(name="ln", bufs=2))
            rowp = phA.enter_context(tc.tile_pool(name="rows", bufs=1))
            ps_t = phA.enter_context(tc.tile_pool(name="psA_t", bufs=2, space="PSUM"))
            ps_mm = phA.enter_context(tc.tile_pool(name="psA_mm", bufs=3, space="PSUM"))
            ps_st = phA.enter_context(tc.tile_pool(name="psA_st", bufs=2, space="PSUM"))

            wlr, _ = load_weightT(wl, stg, ps_t, dram["Wlin_r"], (0.5,), "lr")
            wli, _ = load_weightT(wl, stg, ps_t, dram["Wlin_i"], (0.5, -0.5), "li")
            WlrT, WliT, WliTn = wlr[0.5], wli[0.5], wli[-0.5]

            for sc in range(NSC):
                cr_nat, ci_nat = [], []
                for stl in range(SC // P):
                    s0 = sc * SC + stl * P
                    ld = []
                    for nm in ("raw_real", "bs_real", "raw_imag", "bs_imag"):
                        t = stg.tile([P, D], f32, tag="ld", bufs=6,
                                     name=f"{nm}_{sc}_{stl}")
                        nc.sync.dma_start(out=t, in_=dram[nm][s0:s0 + P, :])
                        ld.append(t)
                    cr = stg.tile([P, D], f32, tag="cn", bufs=8,
                                  name=f"cr_{sc}_{stl}")
                    nc.gpsimd.tensor_tensor(out=cr, in0=ld[0], in1=ld[1],
                                            op=ALU.add)
                    ci = stg.tile([P, D], f32, tag="cn", bufs=8,
                                  name=f"ci_{sc}_{stl}")
                    nc.gpsimd.tensor_tensor(out=ci, in0=ld[2], in1=ld[3],
                                            op=ALU.add)
                    cr_nat.append(cr)
                    ci_nat.append(ci)
                crT, ciT = [], []
                for dt_ in range(DT):
                    for srcl, dst in ((cr_nat, crT), (ci_nat, ciT)):
                        ps = ps_t.tile([P, SC], f32, tag="ps_w", bufs=2,
                                       name=f"ct_ps_{sc}_{dt_}")
                        for stl in range(SC // P):
                            nc.tensor.matmul(
                                ps[:, stl * P:(stl + 1) * P],
                                srcl[stl][:, dt_ * P:(dt_ + 1) * P], ident,
                                is_transpose=True, start=True, stop=True,
                                skip_group_check=True)
                        ct = pair_from(cpool, [P, SC], "cT", 8,
                                       f"cT_{sc}_{dt_}", ps)
                        dst.append(ct)

                # ComplexLinear (x0.5 folded into weights)
                zr_t, zi_t = [], []
                for dt_ in range(DT):
                    dsl = slice(dt_ * P, (dt_ + 1) * P)
                    for wA, zA, wB, zB, lst, pname in (
                            (WlrT, crT, WliTn, ciT, zr_t, "zr"),
                            (WlrT, ciT, WliT, crT, zi_t, "zi")):
                        ps = ps_mm.tile([P, SC], f32, tag="mmA", bufs=3,
                                        name=f"{pname}_ps_{sc}_{dt_}")
                        emit_group(ps,
                                   [(psl(wA[ki], dsl), zA[ki]) for ki in range(DT)]
                                   + [(psl(wB[ki], dsl), zB[ki]) for ki in range(DT)])
                        zt = lnp.tile([P, SC], f32, tag=pname, bufs=4,
                                      name=f"{pname}_{sc}_{dt_}")
                        nc.scalar.copy(out=zt, in_=ps)
                        lst.append(zt)

                # hyp2 = zr^2 + zi^2 ; hypot = sqrt(hyp2)
                h2_t, hp_t = [], []
                for dt_ in range(DT):
                    a = lnp.tile([P, SC], f32, tag="lnA", bufs=4,
                                 name=f"lnA_{sc}_{dt_}")
                    nc.scalar.activation(out=a, in_=zr_t[dt_], func=AF.Square)
                    b = lnp.tile([P, SC], f32, tag="lnB", bufs=2,
                                 name=f"lnB_{sc}_{dt_}")
                    nc.scalar.activation(out=b, in_=zi_t[dt_], func=AF.Square)
                    nc.vector.tensor_add(out=a, in0=a, in1=b)
                    h = lnp.tile([P, SC], f32, tag="lnH", bufs=4,
                                 name=f"lnH_{sc}_{dt_}")
                    nc.scalar.activation(out=h, in_=a, func=AF.Sqrt)
                    h2_t.append(a)
                    hp_t.append(h)

                # stats over partitions via ones-matmuls (fp32 for accuracy)
                ps_sum = ps_st.tile([1, SC], f32, tag="st1", bufs=1,
                                    name=f"sum_ps_{sc}")
                ps_sq = ps_st.tile([1, SC], f32, tag="st2", bufs=1,
                                   name=f"sq_ps_{sc}")
                for dt_ in range(DT):
                    nc.tensor.matmul(ps_sum, ones_col, hp_t[dt_],
                                     start=(dt_ == 0), stop=(dt_ == DT - 1))
                    nc.tensor.matmul(ps_sq, ones_col, h2_t[dt_],
                                     start=(dt_ == 0), stop=(dt_ == DT - 1))
                sum_r = rowp.tile([1, SC], f32, tag="sum_r", name=f"sum_r_{sc}")
                nc.scalar.copy(out=sum_r, in_=ps_sum)
                sq_r = rowp.tile([1, SC], f32, tag="sq_r", name=f"sq_r_{sc}")
                nc.scalar.copy(out=sq_r, in_=ps_sq)
                t0 = rowp.tile([1, SC], f32, tag="t0", name=f"t0_{sc}")
                nc.vector.tensor_mul(out=t0, in0=sum_r, in1=sum_r)
                # t0 = sumsq - sum^2/D  (= var*(D-1))
                nc.vector.scalar_tensor_tensor(out=t0, in0=t0, scalar=-1.0 / D,
                                               in1=sq_r, op0=ALU.mult, op1=ALU.add)
                stdr = rowp.tile([1, SC], f32, tag="stdr", name=f"stdr_{sc}")
                nc.scalar.activation(out=stdr, in_=t0, func=AF.Sqrt,
                                     scale=1.0 / (D - 1), bias=eps_row)
                rstd = rowp.tile([1, SC], f32, tag="rstd", name=f"rstd_{sc}")
                nc.vector.reciprocal(out=rstd, in_=stdr)
                mr = rowp.tile([1, SC], f32, tag="mr", name=f"mr_{sc}")
                nc.vector.scalar_tensor_tensor(out=mr, in0=sum_r, scalar=1.0 / D,
                                               in1=rstd, op0=ALU.mult, op1=ALU.mult)
                rstd_bc = lnp.tile([P, SC], f32, tag="rstd_bc", bufs=2,
                                   name=f"rstd_bc_{sc}")
                nc.gpsimd.partition_broadcast(rstd_bc, rstd)
                mr_bc = lnp.tile([P, SC], f32, tag="mr_bc", bufs=2,
                                 name=f"mr_bc_{sc}")
                nc.gpsimd.partition_broadcast(mr_bc, mr)

                for dt_ in range(DT):
                    H = hp_t[dt_]
                    if trivial_ln and trivial_act:
                        # ratio = rstd - mean*rstd/hypot  (ModReLU == identity)
                        B2 = lnp.tile([P, SC], f32, tag="lnB", bufs=2,
                                      name=f"rh_{sc}_{dt_}")
                        nc.vector.reciprocal(out=B2, in_=H)
                        nc.vector.tensor_mul(out=H, in0=mr_bc, in1=B2)
                        nc.vector.tensor_sub(out=B2, in0=rstd_bc, in1=H)
                        ratio = B2
                    else:
                        NM = lnp.tile([P, SC], f32, tag="gen1", bufs=3,
                                      name=f"nm_{sc}_{dt_}")
                        nc.vector.tensor_mul(out=NM, in0=H, in1=rstd_bc)
                        nc.vector.tensor_sub(out=NM, in0=NM, in1=mr_bc)
                        if not trivial_ln:
                            nc.vector.tensor_scalar(
                                out=NM, in0=NM,
                                scalar1=g_pd[:, dt_:dt_ + 1],
                                scalar2=sh_pd[:, dt_:dt_ + 1],
                                op0=ALU.mult, op1=ALU.add)
                        B2 = lnp.tile([P, SC], f32, tag="lnB", bufs=2,
                                      name=f"rh_{sc}_{dt_}")
                        nc.vector.reciprocal(out=B2, in_=H)
                        ratio = lnp.tile([P, SC], f32, tag="gen2", bufs=3,
                                         name=f"ratio_{sc}_{dt_}")
                        nc.vector.tensor_mul(out=ratio, in0=NM, in1=B2)
                        if not trivial_act:
                            ANM = lnp.tile([P, SC], f32, tag="gen3", bufs=3,
                                           name=f"anm_{sc}_{dt_}")
                            nc.scalar.activation(out=ANM, in_=NM, func=AF.Abs)
                            RL = lnp.tile([P, SC], f32, tag="gen4", bufs=3,
                                          name=f"rl_{sc}_{dt_}")
                            nc.scalar.activation(out=RL, in_=ANM, func=AF.Relu,
                                                 bias=abe_pd[:, dt_:dt_ + 1])
                            nc.vector.tensor_scalar_add(out=ANM, in0=ANM,
                                                        scalar1=EPS)
                            nc.vector.reciprocal(out=ANM, in_=ANM)
                            nc.vector.tensor_mul(out=RL, in0=RL, in1=ANM)
                            nc.vector.tensor_mul(out=ratio, in0=ratio, in1=RL)
                    for zt, lst, pname in ((zr_t[dt_], z2r, "z2r"),
                                           (zi_t[dt_], z2i, "z2i")):
                        if b2:
                            tmp = lnp.tile([P, SC], f32, tag="lnB", bufs=2,
                                           name=f"{pname}t_{sc}_{dt_}")
                            nc.vector.tensor_mul(out=tmp, in0=zt, in1=ratio)
                            z2t = pair_from(zv, [P, SC], "zv", zv_bufs,
                                            f"{pname}_{sc}_{dt_}", tmp)
                        else:
                            z2t = zv.tile([P, SC], mdt, tag="zv", bufs=zv_bufs,
                                          name=f"{pname}_{sc}_{dt_}")
                            nc.vector.tensor_mul(out=z2t, in0=zt, in1=ratio)
                        lst[sc][dt_] = z2t

        # ------------------------------------------------------------------
        # Phase B: q/k/v projections (+ augmented-V head vectors)
        # ------------------------------------------------------------------
        kr_t = [[None] * NSC for _ in range(DT)]
        ki_t = [[None] * NSC for _ in range(DT)]
        vr_t = [None] * ST
        vi_t = [None] * ST
        hv_t = [None] * ST

        with ExitStack() as phB:
            wp = phB.enter_context(tc.tile_pool(name="wqkv", bufs=1))
            stg = phB.enter_context(tc.tile_pool(name="stgB", bufs=6))
            ktp = phB.enter_context(tc.tile_pool(name="ktp", bufs=32))
            hvp = phB.enter_context(tc.tile_pool(name="hvp", bufs=16))
            ps_w = phB.enter_context(tc.tile_pool(name="psB_w", bufs=2, space="PSUM"))
            ps_mm = phB.enter_context(tc.tile_pool(name="psB_mm", bufs=5, space="PSUM"))

            # ---- q: spill transposed q to DRAM ----
            wqr, _ = load_weightT(wp, stg, ps_w, dram["Wq_r"], (1.0,), "qr", ld_bufs=8)
            wqi, _ = load_weightT(wp, stg, ps_w, dram["Wq_i"], (1.0, -1.0), "qi", ld_bufs=8)
            for sc in range(NSC):
                for dt_ in range(DT):
                    dsl = slice(dt_ * P, (dt_ + 1) * P)
                    for wA, zA, wB, zB, comp, pname in (
                            (wqr[1.0], z2r, wqi[-1.0], z2i, "r", "qrs"),
                            (wqr[1.0], z2i, wqi[1.0], z2r, "i", "qis")):
                        ps = ps_mm.tile([P, SC], f32, tag="mmB", bufs=5,
                                        name=f"{pname}_ps_{sc}_{dt_}")
                        emit_group(ps,
                                   [(psl(wA[ki], dsl), zA[sc][ki]) for ki in range(DT)]
                                   + [(psl(wB[ki], dsl), zB[sc][ki]) for ki in range(DT)])
                        st_t = pair_from(stg, [P, SC], "qstg", 4,
                                         f"{pname}_{sc}_{dt_}", ps)
                        if b2:
                            nc.sync.dma_start(
                                out=qsp[(comp, "h")][dsl, sc * SC:(sc + 1) * SC],
                                in_=st_t[0])
                            nc.sync.dma_start(
                                out=qsp[(comp, "l")][dsl, sc * SC:(sc + 1) * SC],
                                in_=st_t[1])
                        else:
                            nc.sync.dma_start(
                                out=qsp[(comp, "")][dsl, sc * SC:(sc + 1) * SC],
                                in_=st_t)

            # ---- k: keep transposed k resident ----
            wkr, _ = load_weightT(wp, stg, ps_w, dram["Wk_r"], (1.0,), "kr", ld_bufs=8)
            wki, _ = load_weightT(wp, stg, ps_w, dram["Wk_i"], (1.0, -1.0), "ki", ld_bufs=8)
            for sc in range(NSC):
                for dt_ in range(DT):
                    dsl = slice(dt_ * P, (dt_ + 1) * P)
                    for wA, zA, wB, zB, dst, pname in (
                            (wkr[1.0], z2r, wki[-1.0], z2i, kr_t, "krs"),
                            (wkr[1.0], z2i, wki[1.0], z2r, ki_t, "kis")):
                        ps = ps_mm.tile([P, SC], f32, tag="mmB", bufs=5,
                                        name=f"{pname}_ps_{sc}_{dt_}")
                        emit_group(ps,
                                   [(psl(wA[ki], dsl), zA[sc][ki]) for ki in range(DT)]
                                   + [(psl(wB[ki], dsl), zB[sc][ki]) for ki in range(DT)])
                        dst[dt_][sc] = pair_from(ktp, [P, SC], "kt", 32,
                                                 f"{pname}_{sc}_{dt_}", ps)

            # ---- v (natural layout) + head vectors ----
            wvr, nat_vr = load_weightT(wp, stg, ps_w, dram["Wv_r"], (1.0,), "vr", ld_bufs=8)
            wvi, nat_vi = load_weightT(wp, stg, ps_w, dram["Wv_i"], (1.0, -1.0), "vi", ld_bufs=8)

            # fh[, t, h] = filt * W_head_half   (r and i halves + negated r)
            fh_r = const.tile([P, DT, 3], f32, tag="fh_r", name="fh_r")
            fh_i = const.tile([P, DT, 3], f32, tag="fh_i", name="fh_i")
            for h, nm in enumerate(HEAD_W):
                nc.sync.dma_start(
                    out=fh_r[:, :, h:h + 1],
                    in_=dram[nm][:, 0:D].rearrange("o (t p) -> p t o", p=P))
                nc.sync.dma_start(
                    out=fh_i[:, :, h:h + 1],
                    in_=dram[nm][:, D:2 * D].rearrange("o (t p) -> p t o", p=P))
            for t in range(DT):
                nc.vector.tensor_scalar_mul(out=fh_r[:, t, :], in0=fh_r[:, t, :],
                                            scalar1=filt_pd[:, t:t + 1])
                nc.vector.tensor_scalar_mul(out=fh_i[:, t, :], in0=fh_i[:, t, :],
                                            scalar1=filt_pd[:, t:t + 1])
            fh_rn = const.tile([P, DT, 3], f32, tag="fh_rn", name="fh_rn")
            nc.scalar.mul(out=fh_rn, in_=fh_r, mul=-1.0)

            # u_r[di, h] = Wvr_nat.T @ fh_r + Wvi_nat.T @ fh_i
            # u_i[di, h] = Wvr_nat.T @ fh_i - Wvi_nat.T @ fh_r
            u_r = [None] * DT
            u_i = [None] * DT
            for ti in range(DT):
                tsl = slice(ti * P, (ti + 1) * P)
                for natA, rhsA, natB, rhsB, dst, pname in (
                        (nat_vr, fh_r, nat_vi, fh_i, u_r, "u_r"),
                        (nat_vr, fh_i, nat_vi, fh_rn, u_i, "u_i")):
                    ps = ps_w.tile([P, 3], f32, tag="ps_w", bufs=2,
                                   name=f"{pname}_ps_{ti}")
                    for to in range(DT):
                        nc.tensor.matmul(ps, natA[to][:, tsl],
                                         rhsA[:, to, :], start=(to == 0),
                                         stop=False)
                    for to in range(DT):
                        nc.tensor.matmul(ps, natB[to][:, tsl],
                                         rhsB[:, to, :], start=False,
                                         stop=(to == DT - 1))
                    dst[ti] = pair_from(const, [P, 3], f"{pname}{ti}", 1,
                                        f"{pname}_{ti}", ps)

            for sc in range(NSC):
                ps_v = []
                for stl in range(SC // P):
                    st = sc * (SC // P) + stl
                    sl = slice(stl * P, (stl + 1) * P)
                    for zA, wA, zB, wB, pname in (
                            (z2r, wvr[1.0], z2i, wvi[-1.0], "vrs"),
                            (z2i, wvr[1.0], z2r, wvi[1.0], "vis")):
                        ps = ps_mm.tile([P, D], f32, tag="mmB", bufs=5,
                                        name=f"{pname}_ps_{st}")
                        emit_group(ps,
                                   [(psl(zA[sc][ki], sl), wA[ki]) for ki in range(DT)]
                                   + [(psl(zB[sc][ki], sl), wB[ki]) for ki in range(DT)])
                        ps_v.append(ps)
                    psh = ps_w.tile([P, 3], f32, tag="ps_w", bufs=2,
                                    name=f"hv_ps_{st}")
                    if b2:
                        emit_group(psh,
                                   [(psl(z2r[sc][ki], sl), u_r[ki]) for ki in range(DT)]
                                   + [(psl(z2i[sc][ki], sl), u_i[ki]) for ki in range(DT)])
                    else:
                        for ki in range(DT):
                            nc.tensor.matmul(psh, z2r[sc][ki][:, sl].bitcast(f32),
                                             u_r[ki], start=(ki == 0), stop=False)
                        for ki in range(DT):
                            nc.tensor.matmul(psh, z2i[sc][ki][:, sl].bitcast(f32),
                                             u_i[ki], start=False,
                                             stop=(ki == DT - 1))
                    if b2:
                        hvh = hvp.tile([P, 4], bf16, tag="hvh", bufs=16,
                                       name=f"hvh_{st}")
                        nc.scalar.copy(out=hvh[:, 0:1], in_=ones_col)
                        nc.scalar.copy(out=hvh[:, 1:4], in_=psh)
                        hvl = hvp.tile([P, 4], bf16, tag="hvl", bufs=16,
                                       name=f"hvl_{st}")
                        nc.gpsimd.memset(hvl[:, 0:1], 0.0)
                        nc.vector.scalar_tensor_tensor(
                            out=hvl[:, 1:4], in0=hvh[:, 1:4], scalar=-1.0,
                            in1=psh, op0=ALU.mult, op1=ALU.add)
                        hv_t[st] = (hvh, hvl)
                    else:
                        hv = hvp.tile([P, 4], f32, tag="hv", bufs=16,
                                      name=f"hv_{st}")
                        nc.scalar.copy(out=hv[:, 0:1], in_=ones_col)
                        nc.scalar.copy(out=hv[:, 1:4], in_=psh)
                        hv_t[st] = hv
                # copybacks (v = psum * filt_bc); reuse z2 slots in zv pool
                for stl in range(SC // P):
                    st = sc * (SC // P) + stl
                    for j, (lst, pname) in enumerate(((vr_t, "vrf"),
                                                      (vi_t, "vif"))):
                        psv = ps_v[2 * stl + j]
                        if b2:
                            tmp = stg.tile([P, D], f32, tag="vtmp", bufs=2,
                                           name=f"{pname}t_{st}")
                            nc.vector.tensor_tensor(out=tmp, in0=psv,
                                                    in1=filt_bc, op=ALU.mult)
                            lst[st] = pair_from(zv, [P, D], "zv", zv_bufs,
                                                f"{pname}_{st}", tmp)
                        else:
                            v = zv.tile([P, D], mdt, tag="zv", bufs=zv_bufs,
                                        name=f"{pname}_{st}")
                            nc.vector.tensor_tensor(out=v, in0=psv,
                                                    in1=filt_bc, op=ALU.mult)
                            lst[st] = v

        # ------------------------------------------------------------------
        # Phase C: attention (scoresT -> exp -> attn@[v|heads] -> outputs)
        # ------------------------------------------------------------------
        with ExitStack() as phC:
            qp = phC.enter_context(tc.tile_pool(name="qp", bufs=16))
            ep = phC.enter_context(tc.tile_pool(name="ep", bufs=18))
            opool = phC.enter_context(tc.tile_pool(name="outp", bufs=3))
            hp = phC.enter_context(tc.tile_pool(name="headp", bufs=1))
            ps_s = phC.enter_context(tc.tile_pool(name="psC_s", bufs=2, space="PSUM"))
            ps_o = phC.enter_context(tc.tile_pool(name="psC_o", bufs=4, space="PSUM"))
            ps_h = phC.enter_context(tc.tile_pool(name="psC_h", bufs=2, space="PSUM"))

            heads_s = hp.tile([P, ST], f32, tag="hs", name="heads_s")
            heads_c = hp.tile([P, ST], f32, tag="hc", name="heads_c")
            heads_h = hp.tile([P, ST], f32, tag="hh", name="heads_h")
            scale = float(D ** -0.5)

            for qc in range(NQC):
                qr_c, qi_c = [], []
                for comp, lst in (("r", qr_c), ("i", qi_c)):
                    for dt_ in range(DT):
                        dsl = slice(dt_ * P, (dt_ + 1) * P)
                        csl = slice(qc * QC, (qc + 1) * QC)
                        if b2:
                            th = qp.tile([P, QC], bf16, tag="qTh", bufs=16,
                                         name=f"q{comp}h_c{qc}_{dt_}")
                            nc.sync.dma_start(out=th, in_=qsp[(comp, "h")][dsl, csl])
                            tl = qp.tile([P, QC], bf16, tag="qTl", bufs=16,
                                         name=f"q{comp}l_c{qc}_{dt_}")
                            nc.sync.dma_start(out=tl, in_=qsp[(comp, "l")][dsl, csl])
                            lst.append((th, tl))
                        else:
                            t = qp.tile([P, QC], mdt, tag="qT", bufs=16,
                                        name=f"q{comp}_c{qc}_{dt_}")
                            nc.sync.dma_start(out=t, in_=qsp[(comp, "")][dsl, csl])
                            lst.append(t)
                exps = []
                for kt in range(KT):
                    ps = ps_s.tile([P, QC], f32, tag="ps_s", bufs=2,
                                   name=f"sc_ps_{qc}_{kt}")
                    kq = SC // P
                    ksl = slice((kt % kq) * P, (kt % kq) * P + P)
                    ksc = kt // kq
                    emit_group(ps,
                               [(psl(kr_t[ki][ksc], ksl), qr_c[ki]) for ki in range(DT)]
                               + [(psl(ki_t[ki][ksc], ksl), qi_c[ki]) for ki in range(DT)])
                    if b2:
                        e32 = ep.tile([P, QC], f32, tag="e32", bufs=3,
                                      name=f"e32_{qc}_{kt}")
                        nc.scalar.activation(out=e32, in_=ps, func=AF.Exp,
                                             scale=scale)
                        e = pair_from(ep, [P, QC], "exp", 18,
                                      f"exp_{qc}_{kt}", e32)
                    else:
                        e = ep.tile([P, QC], mdt, tag="exp", bufs=18,
                                    name=f"exp_{qc}_{kt}")
                        nc.scalar.activation(out=e, in_=ps, func=AF.Exp,
                                             scale=scale)
                    exps.append(e)
                for qt in range(QC // P):
                    qti = qc * (QC // P) + qt
                    s0 = qti * P
                    pso_r = ps_o.tile([P, D], f32, tag="ps_o", bufs=4,
                                      name=f"or_ps_{qti}")
                    pso_i = ps_o.tile([P, D], f32, tag="ps_o", bufs=4,
                                      name=f"oi_ps_{qti}")
                    psh = ps_h.tile([P, 4], f32, tag="ps_h", bufs=2,
                                    name=f"h_ps_{qti}")
                    qsl = slice(qt * P, (qt + 1) * P)
                    emit_group(pso_r, [(psl(exps[kt], qsl), vr_t[kt])
                                       for kt in range(KT)])
                    emit_group(pso_i, [(psl(exps[kt], qsl), vi_t[kt])
                                       for kt in range(KT)])
                    if b2:
                        emit_group(psh, [(psl(exps[kt], qsl), hv_t[kt])
                                         for kt in range(KT)])
                    else:
                        for kt in range(KT):
                            nc.tensor.matmul(psh, exps[kt][:, qsl].bitcast(f32),
                                             hv_t[kt], start=(kt == 0),
                                             stop=(kt == KT - 1))
                    rc = opool.tile([P, 1], f32, tag="recip", bufs=4,
                                    name=f"rc_{qti}")
                    nc.vector.reciprocal(out=rc, in_=psh[:, 0:1])
                    prt = opool.tile([P, D], f32, tag="pr", bufs=3,
                                     name=f"prt_{qti}")
                    nc.vector.tensor_scalar_mul(out=prt, in0=pso_r, scalar1=rc)
                    nc.sync.dma_start(out=out_pr[s0:s0 + P, :], in_=prt)
                    pit = opool.tile([P, D], f32, tag="pr", bufs=3,
                                     name=f"pit_{qti}")
                    nc.vector.tensor_scalar_mul(out=pit, in0=pso_i, scalar1=rc)
                    nc.sync.dma_start(out=out_pi[s0:s0 + P, :], in_=pit)
                    nc.vector.tensor_scalar(out=heads_s[:, qti:qti + 1],
                                            in0=psh[:, 1:2], scalar1=rc,
                                            scalar2=b_score_bc, op0=ALU.mult,
                                            op1=ALU.add)
                    nc.scalar.activation(out=heads_c[:, qti:qti + 1],
                                         in_=psh[:, 2:3], func=AF.Sigmoid,
                                         scale=rc, bias=b_conf_bc)
                    nc.vector.tensor_scalar(out=heads_h[:, qti:qti + 1],
                                            in0=psh[:, 3:4], scalar1=rc,
                                            scalar2=b_halt_bc, op0=ALU.mult,
                                            op1=ALU.add)

            # heads: [P, ST] -> transpose -> [ST, P] -> contiguous DMA
            for src, dst in ((heads_s, out_sc), (heads_c, out_cf),
                             (heads_h, out_hl)):
                pst = ps_h.tile([ST, P], f32, tag="ps_h", bufs=2,
                                name=f"hT_ps_{src.name}")
                nc.tensor.matmul(pst, src, ident, is_transpose=True,
                                 start=True, stop=True, skip_group_check=True)
                sb = opool.tile([ST, P], f32, tag="hT", bufs=2,
                                name=f"hT_{src.name}")
                nc.scalar.copy(out=sb, in_=pst)
                nc.sync.dma_start(
                    out=dst.rearrange("(t p) o -> t (p o)", p=P), in_=sb)

    nc.compile()
    return nc


_CACHE = {}


def _get_program(S, mm_mode, trivial_ln, trivial_act):
    key = (S, mm_mode, trivial_ln, trivial_act)
    if key not in _CACHE:
        _CACHE[key] = build_program(S, mm_mode, trivial_ln, trivial_act)
    return _CACHE[key]


def kernel(**inputs):
    inputs = {k: np.asarray(v, dtype=np.float32) if np.asarray(v).dtype != np.float32
              else np.asarray(v) for k, v in inputs.items()}
    B, S, D_ = inputs["raw_real"].shape
    assert B == N_CORES and D_ == D
    trivial_ln = bool(np.all(inputs["ln_scale"] == 1.0)
                      and np.all(inputs["ln_shift"] == 0.0))
    trivial_act = bool(np.all(inputs["act_bias"] == 0.0))
    mm_mode = os.environ.get("MM_MODE", "b2")
    nc = _get_program(S, mm_mode, trivial_ln, trivial_act)

    shared = {}
    for nm in W_NAMES + VEC_NAMES + HEAD_W + HEAD_B:
        shared[nm] = np.ascontiguousarray(inputs[nm])
    in_maps = []
    for b in range(B):
        m = dict(shared)
        for nm in ("raw_real", "raw_imag", "bs_real", "bs_imag"):
            m[nm] = np.ascontiguousarray(inputs[nm][b])
        in_maps.append(m)

    res = run_bass_kernel_spmd(nc, in_maps, core_ids=list(range(N_CORES)))
    r = res.results
    pr = np.stack([r[b]["pr"] for b in range(B)])
    pi = np.stack([r[b]["pi"] for b in range(B)])
    score = np.stack([r[b]["score"] for b in range(B)])
    conf = np.stack([r[b]["confidence"] for b in range(B)])
    halt = np.stack([r[b]["halt"] for b in range(B)])
    filt = r[0]["filt"]
    return (pr, pi, score, conf, halt, filt)


if __name__ == "__main__":
    nc = build_program(S=512)
    print("built ok")


# revision 17
# speedup vs baseline: 4.8284x; 4.8284x over previous
"""Trainium2 Bass kernel for nn_ComplexAttentionBlock.

Sharding: data-parallel over batch B=8, one sample per NeuronCore.
Per-core layout strategy:
  - activations kept in transposed ("T") layout [feature, seq] so every
    matmul contracts over the partition dim with zero activation
    transposes after the initial c -> cT transpose.
  - LayerNorm row stats (over features = partitions in T layout) via
    ones-vector matmuls on the PE; broadcast back via gpsimd
    partition_broadcast.
  - attention scores computed directly transposed [kpos, q] so the
    exp() output is directly the lhsT of the attn@v matmul.
  - rowsum + the three heads (score/conf/halt) folded into a single
    [kpos, 4] augmented-V matmul: col0 = ones (rowsum), cols 1..3 =
    V projected through filt*W_head.
  - softmax skips max-subtraction (scores max ~8.5, exp is safe).
  - mm_mode "b2": every heavy matmul operand is split into a bf16
    hi/lo pair; each product becomes 3 bf16 matmuls (hi*hi + hi*lo +
    lo*hi) accumulating in the same PSUM group.  3 cyc/row vs fp32's
    4 cyc/row, with ~2^-16 operand precision.
"""

import os
import numpy as np
from contextlib import ExitStack

import concourse.bass as bass
import concourse.bacc as bacc
import concourse.mybir as mybir
import concourse.tile as tile
from concourse.bass_utils import run_bass_kernel_spmd
from concourse.masks import make_identity

f32 = mybir.dt.float32
f32r = mybir.dt.float32r
bf16 = mybir.dt.bfloat16
AF = mybir.ActivationFunctionType
ALU = mybir.AluOpType

P = 128
D = 512
DT = D // P  # 4 feature tiles
EPS = 1e-6
N_CORES = 8

W_NAMES = ("Wlin_r", "Wlin_i", "Wq_r", "Wq_i", "Wk_r", "Wk_i", "Wv_r", "Wv_i")
VEC_NAMES = ("ln_scale", "ln_shift", "act_bias", "gate_mask")
HEAD_W = ("W_score", "W_conf", "W_halt")
HEAD_B = ("b_score", "b_conf", "b_halt")


def build_program(S=2048, mm_mode="b2", trivial_ln=True, trivial_act=True):
    SC = 512
    NSC = S // SC          # s-chunks for phases A/B
    ST = S // P            # s-tiles (also kpos tiles)
    QC = 256
    NQC = S // QC          # q-chunks for attention
    KT = ST
    assert S % SC == 0 and S % QC == 0
    b2 = (mm_mode == "b2")
    mdt = f32r if mm_mode == "f32r" else f32

    nc = bacc.Bacc("TRN2", target_bir_lowering=False, debug=False,
                   num_devices=N_CORES)

    dram = {}
    for nm in ("raw_real", "raw_imag", "bs_real", "bs_imag"):
        dram[nm] = nc.dram_tensor(nm, [S, D], f32, kind="ExternalInput").ap()
    for nm in W_NAMES:
        dram[nm] = nc.dram_tensor(nm, [D, D], f32, kind="ExternalInput").ap()
    for nm in VEC_NAMES:
        dram[nm] = nc.dram_tensor(nm, [D], f32, kind="ExternalInput").ap()
    for nm in HEAD_W:
        dram[nm] = nc.dram_tensor(nm, [1, 2 * D], f32, kind="ExternalInput").ap()
    for nm in HEAD_B:
        dram[nm] = nc.dram_tensor(nm, [1], f32, kind="ExternalInput").ap()

    out_pr = nc.dram_tensor("pr", [S, D], f32, kind="ExternalOutput").ap()
    out_pi = nc.dram_tensor("pi", [S, D], f32, kind="ExternalOutput").ap()
    out_sc = nc.dram_tensor("score", [S, 1], f32, kind="ExternalOutput").ap()
    out_cf = nc.dram_tensor("confidence", [S, 1], f32, kind="ExternalOutput").ap()
    out_hl = nc.dram_tensor("halt", [S, 1], f32, kind="ExternalOutput").ap()
    out_ft = nc.dram_tensor("filt", [D], f32, kind="ExternalOutput").ap()

    with tile.TileContext(nc) as tc, ExitStack() as top:
        const = top.enter_context(tc.tile_pool(name="const", bufs=1))
        zv = top.enter_context(tc.tile_pool(name="zv", bufs=35))
        dsp = top.enter_context(tc.tile_pool(name="dspill", bufs=1, space="DRAM"))
        zv_bufs = 36 if b2 else 35

        # ---------- pair helpers ----------
        def pair_from(pool, shape, tag, bufs, name, src, sign=1.0):
            """Copy src (f32 PSUM/SBUF AP) into a matmul operand: a bf16
            hi/lo pair in b2 mode, else a single mdt tile."""
            if not b2:
                t = pool.tile(shape, mdt, tag=tag, bufs=bufs, name=name)
                if sign == 1.0:
                    nc.scalar.copy(out=t, in_=src)
                else:
                    nc.scalar.mul(out=t, in_=src, mul=float(sign))
                return t
            hi = pool.tile(shape, bf16, tag=tag + "h", bufs=bufs,
                           name=name + "_h")
            if sign == 1.0:
                nc.scalar.copy(out=hi, in_=src)
            else:
                nc.scalar.mul(out=hi, in_=src, mul=float(sign))
            lo = pool.tile(shape, bf16, tag=tag + "l", bufs=bufs,
                           name=name + "_l")
            # lo = sign*src - hi:  (hi * -1) +/- src
            nc.vector.scalar_tensor_tensor(
                out=lo, in0=hi, scalar=-1.0, in1=src, op0=ALU.mult,
                op1=ALU.add if sign > 0 else ALU.subtract)
            return (hi, lo)

        def psl(x, sl):
            if b2:
                return (x[0][:, sl], x[1][:, sl])
            return x[:, sl]

        def opr(x):
            """matmul-ready view of a single (non-pair) operand."""
            return x.bitcast(f32r) if mm_mode == "f32r" else x

        def emit_group(ps, terms):
            mml = []
            for l, r in terms:
                if b2:
                    mml += [(l[0], r[0]), (l[0], r[1]), (l[1], r[0])]
                else:
                    mml.append((opr(l), opr(r)))
            n = len(mml)
            for i, (l, r) in enumerate(mml):
                nc.tensor.matmul(ps, l, r, start=(i == 0), stop=(i == n - 1))

        qsp = {}
        qkeys = [("r", "h"), ("r", "l"), ("i", "h"), ("i", "l")] if b2 \
            else [("r", ""), ("i", "")]
        for c, h in qkeys:
            qsp[(c, h)] = dsp.tile([D, S], bf16 if b2 else mdt,
                                   tag=f"qsp_{c}{h}", name=f"qsp_{c}{h}")

        ident = const.tile([P, P], f32, tag="ident", name="ident")
        make_identity(nc, ident)
        ones_col = const.tile([P, 1], f32, tag="ones_col", name="ones_col")
        nc.vector.memset(ones_col, 1.0)
        eps_row = const.tile([1, 1], f32, tag="eps_row", name="eps_row")
        nc.vector.memset(eps_row, EPS)

        # gate -> filt, in both [P, DT] (per-partition) and [1, D] (row) forms
        gate_pd = const.tile([P, DT], f32, tag="gate_pd", name="gate_pd")
        nc.sync.dma_start(out=gate_pd,
                          in_=dram["gate_mask"].rearrange("(t p) -> p t", p=P))
        filt_pd = const.tile([P, DT], f32, tag="filt_pd", name="filt_pd")
        nc.scalar.activation(out=filt_pd, in_=gate_pd, func=AF.Sigmoid)
        gate_row = const.tile([1, D], f32, tag="gate_row", name="gate_row")
        nc.sync.dma_start(out=gate_row,
                          in_=dram["gate_mask"].rearrange("(o d) -> o d", o=1))
        filt_row = gate_row
        nc.scalar.activation(out=filt_row, in_=gate_row, func=AF.Sigmoid)
        nc.sync.dma_start(out=out_ft.rearrange("(o d) -> o d", o=1), in_=filt_row)
        filt_bc = const.tile([P, D], f32, tag="filt_bc", name="filt_bc")
        nc.gpsimd.partition_broadcast(filt_bc, filt_row)

        def load_scalar_bc(nm):
            r = const.tile([1, 1], f32, tag=nm + "_r", name=nm + "_r")
            nc.sync.dma_start(out=r, in_=dram[nm].rearrange("(o d) -> o d", o=1))
            t = const.tile([P, 1], f32, tag=nm + "_bc", name=nm + "_bc")
            nc.gpsimd.partition_broadcast(t, r)
            return t

        b_score_bc = load_scalar_bc("b_score")
        b_conf_bc = load_scalar_bc("b_conf")
        b_halt_bc = load_scalar_bc("b_halt")

        if not trivial_ln:
            g_pd = const.tile([P, DT], f32, tag="g_pd", name="g_pd")
            nc.sync.dma_start(out=g_pd,
                              in_=dram["ln_scale"].rearrange("(t p) -> p t", p=P))
            sh_pd = const.tile([P, DT], f32, tag="sh_pd", name="sh_pd")
            nc.sync.dma_start(out=sh_pd,
                              in_=dram["ln_shift"].rearrange("(t p) -> p t", p=P))
        if not trivial_act:
            ab_pd = const.tile([P, DT], f32, tag="ab_pd", name="ab_pd")
            nc.sync.dma_start(out=ab_pd,
                              in_=dram["act_bias"].rearrange("(t p) -> p t", p=P))
            abe_pd = const.tile([P, DT], f32, tag="abe_pd", name="abe_pd")
            nc.vector.tensor_scalar_add(out=abe_pd, in0=ab_pd, scalar1=EPS)

        def load_weightT(wpool, stg, pspool, dram_ap, scales, nm, ld_bufs=6):
            """Load W [D, D] (torch layout [out, in]) and produce transposed
            operand tiles WT[di_tile] scaled by each s in scales."""
            nat = []
            for t in range(DT):
                wn = stg.tile([P, D], f32, tag="ld", bufs=ld_bufs,
                              name=f"{nm}_nat{t}")
                nc.sync.dma_start(out=wn, in_=dram_ap[t * P:(t + 1) * P, :])
                nat.append(wn)
            outs = {s: [] for s in scales}
            for ti in range(DT):
                ps = pspool.tile([P, D], f32, tag="ps_w", bufs=2,
                                 name=f"{nm}_ps{ti}")
                for to in range(DT):
                    nc.tensor.matmul(ps[:, to * P:(to + 1) * P],
                                     nat[to][:, ti * P:(ti + 1) * P], ident,
                                     is_transpose=True, start=True, stop=True,
                                     skip_group_check=True)
                for s in scales:
                    bufs = 8 if s > 0 else 4
                    if b2 and abs(s) != 1.0:
                        tmp = stg.tile([P, D], f32, tag="cn", bufs=8,
                                       name=f"{nm}_tmp_{s}_{ti}")
                        nc.scalar.mul(out=tmp, in_=ps, mul=float(s))
                        wt = pair_from(wpool, [P, D], f"w{s}", bufs,
                                       f"{nm}T_{s}_{ti}", tmp, sign=1.0)
                    else:
                        wt = pair_from(wpool, [P, D], f"w{s}", bufs,
                                       f"{nm}T_{s}_{ti}", ps, sign=s)
                    outs[s].append(wt)
            return outs, nat

        z2r = [[None] * DT for _ in range(NSC)]
        z2i = [[None] * DT for _ in range(NSC)]

        # ------------------------------------------------------------------
        # Phase A: combine -> transpose -> ComplexLinear -> LN+ModReLU -> z2T
        # ------------------------------------------------------------------
        with ExitStack() as phA:
            wl = phA.enter_context(tc.tile_pool(name="wlin", bufs=1))
            stg = phA.enter_context(tc.tile_pool(name="stgA", bufs=8))
            cpool = phA.enter_context(tc.tile_pool(name="cT", bufs=8))
            lnp = phA.enter_context(tc.tile_pool(name="ln", bufs=2))
            rowp = phA.enter_context(tc.tile_pool(name="rows", bufs=1))
            ps_t = phA.enter_context(tc.tile_pool(name="psA_t", bufs=2, space="PSUM"))
            ps_mm = phA.enter_context(tc.tile_pool(name="psA_mm", bufs=4, space="PSUM"))
            ps_st = phA.enter_context(tc.tile_pool(name="psA_st", bufs=2, space="PSUM"))

            wlr, _ = load_weightT(wl, stg, ps_t, dram["Wlin_r"], (0.5,), "lr")
            wli, _ = load_weightT(wl, stg, ps_t, dram["Wlin_i"], (0.5, -0.5), "li")
            WlrT, WliT, WliTn = wlr[0.5], wli[0.5], wli[-0.5]

            for sc in range(NSC):
                cr_nat, ci_nat = [], []
                for stl in range(SC // P):
                    s0 = sc * SC + stl * P
                    ld = []
                    for nm in ("raw_real", "bs_real", "raw_imag", "bs_imag"):
                        t = stg.tile([P, D], f32, tag="ld", bufs=6,
                                     name=f"{nm}_{sc}_{stl}")
                        nc.sync.dma_start(out=t, in_=dram[nm][s0:s0 + P, :])
                        ld.append(t)
                    cr = stg.tile([P, D], f32, tag="cn", bufs=8,
                                  name=f"cr_{sc}_{stl}")
                    nc.gpsimd.tensor_tensor(out=cr, in0=ld[0], in1=ld[1],
                                            op=ALU.add)
                    ci = stg.tile([P, D], f32, tag="cn", bufs=8,
                                  name=f"ci_{sc}_{stl}")
                    nc.gpsimd.tensor_tensor(out=ci, in0=ld[2], in1=ld[3],
                                            op=ALU.add)
                    cr_nat.append(cr)
                    ci_nat.append(ci)
                crT, ciT = [], []
                for dt_ in range(DT):
                    for srcl, dst in ((cr_nat, crT), (ci_nat, ciT)):
                        ps = ps_t.tile([P, SC], f32, tag="ps_w", bufs=2,
                                       name=f"ct_ps_{sc}_{dt_}")
                        for stl in range(SC // P):
                            nc.tensor.matmul(
                                ps[:, stl * P:(stl + 1) * P],
                                srcl[stl][:, dt_ * P:(dt_ + 1) * P], ident,
                                is_transpose=True, start=True, stop=True,
                                skip_group_check=True)
                        ct = pair_from(cpool, [P, SC], "cT", 8,
                                       f"cT_{sc}_{dt_}", ps)
                        dst.append(ct)

                # ComplexLinear (x0.5 folded into weights)
                zr_t, zi_t = [], []
                for dt_ in range(DT):
                    dsl = slice(dt_ * P, (dt_ + 1) * P)
                    for wA, zA, wB, zB, lst, pname in (
                            (WlrT, crT, WliTn, ciT, zr_t, "zr"),
                            (WlrT, ciT, WliT, crT, zi_t, "zi")):
                        ps = ps_mm.tile([P, SC], f32, tag="mmA", bufs=4,
                                        name=f"{pname}_ps_{sc}_{dt_}")
                        emit_group(ps,
                                   [(psl(wA[ki], dsl), zA[ki]) for ki in range(DT)]
                                   + [(psl(wB[ki], dsl), zB[ki]) for ki in range(DT)])
                        zt = lnp.tile([P, SC], f32, tag=pname, bufs=4,
                                      name=f"{pname}_{sc}_{dt_}")
                        nc.scalar.copy(out=zt, in_=ps)
                        lst.append(zt)

                # hyp2 = zr^2 + zi^2 ; hypot = sqrt(hyp2)
                h2_t, hp_t = [], []
                for dt_ in range(DT):
                    a = lnp.tile([P, SC], f32, tag="lnA", bufs=4,
                                 name=f"lnA_{sc}_{dt_}")
                    nc.scalar.activation(out=a, in_=zr_t[dt_], func=AF.Square)
                    b = lnp.tile([P, SC], f32, tag="lnB", bufs=2,
                                 name=f"lnB_{sc}_{dt_}")
                    nc.scalar.activation(out=b, in_=zi_t[dt_], func=AF.Square)
                    nc.vector.tensor_add(out=a, in0=a, in1=b)
                    h = lnp.tile([P, SC], f32, tag="lnH", bufs=4,
                                 name=f"lnH_{sc}_{dt_}")
                    nc.scalar.activation(out=h, in_=a, func=AF.Sqrt)
                    h2_t.append(a)
                    hp_t.append(h)

                # stats over partitions via ones-matmuls (fp32 for accuracy)
                ps_sum = ps_st.tile([1, SC], f32, tag="st1", bufs=1,
                                    name=f"sum_ps_{sc}")
                ps_sq = ps_st.tile([1, SC], f32, tag="st2", bufs=1,
                                   name=f"sq_ps_{sc}")
                for dt_ in range(DT):
                    nc.tensor.matmul(ps_sum, ones_col, hp_t[dt_],
                                     start=(dt_ == 0), stop=(dt_ == DT - 1))
                    nc.tensor.matmul(ps_sq, ones_col, h2_t[dt_],
                                     start=(dt_ == 0), stop=(dt_ == DT - 1))
                sum_r = rowp.tile([1, SC], f32, tag="sum_r", name=f"sum_r_{sc}")
                nc.scalar.copy(out=sum_r, in_=ps_sum)
                sq_r = rowp.tile([1, SC], f32, tag="sq_r", name=f"sq_r_{sc}")
                nc.scalar.copy(out=sq_r, in_=ps_sq)
                t0 = rowp.tile([1, SC], f32, tag="t0", name=f"t0_{sc}")
                nc.vector.tensor_mul(out=t0, in0=sum_r, in1=sum_r)
                # t0 = sumsq - sum^2/D  (= var*(D-1))
                nc.vector.scalar_tensor_tensor(out=t0, in0=t0, scalar=-1.0 / D,
                                               in1=sq_r, op0=ALU.mult, op1=ALU.add)
                stdr = rowp.tile([1, SC], f32, tag="stdr", name=f"stdr_{sc}")
                nc.scalar.activation(out=stdr, in_=t0, func=AF.Sqrt,
                                     scale=1.0 / (D - 1), bias=eps_row)
                rstd = rowp.tile([1, SC], f32, tag="rstd", name=f"rstd_{sc}")
                nc.vector.reciprocal(out=rstd, in_=stdr)
                mr = rowp.tile([1, SC], f32, tag="mr", name=f"mr_{sc}")
                nc.vector.scalar_tensor_tensor(out=mr, in0=sum_r, scalar=1.0 / D,
                                               in1=rstd, op0=ALU.mult, op1=ALU.mult)
                rstd_bc = lnp.tile([P, SC], f32, tag="rstd_bc", bufs=2,
                                   name=f"rstd_bc_{sc}")
                nc.gpsimd.partition_broadcast(rstd_bc, rstd)
                mr_bc = lnp.tile([P, SC], f32, tag="mr_bc", bufs=2,
                                 name=f"mr_bc_{sc}")
                nc.gpsimd.partition_broadcast(mr_bc, mr)

                for dt_ in range(DT):
                    H = hp_t[dt_]
                    if trivial_ln and trivial_act:
                        # ratio = rstd - mean*rstd/hypot  (ModReLU == identity)
                        B2 = lnp.tile([P, SC], f32, tag="lnB", bufs=2,
                                      name=f"rh_{sc}_{dt_}")
                        nc.vector.reciprocal(out=B2, in_=H)
                        nc.vector.tensor_mul(out=H, in0=mr_bc, in1=B2)
                        nc.vector.tensor_sub(out=B2, in0=rstd_bc, in1=H)
                        ratio = B2
                    else:
                        NM = lnp.tile([P, SC], f32, tag="gen1", bufs=3,
                                      name=f"nm_{sc}_{dt_}")
                        nc.vector.tensor_mul(out=NM, in0=H, in1=rstd_bc)
                        nc.vector.tensor_sub(out=NM, in0=NM, in1=mr_bc)
                        if not trivial_ln:
                            nc.vector.tensor_scalar(
                                out=NM, in0=NM,
                                scalar1=g_pd[:, dt_:dt_ + 1],
                                scalar2=sh_pd[:, dt_:dt_ + 1],
                                op0=ALU.mult, op1=ALU.add)
                        B2 = lnp.tile([P, SC], f32, tag="lnB", bufs=2,
                                      name=f"rh_{sc}_{dt_}")
                        nc.vector.reciprocal(out=B2, in_=H)
                        ratio = lnp.tile([P, SC], f32, tag="gen2", bufs=3,
                                         name=f"ratio_{sc}_{dt_}")
                        nc.vector.tensor_mul(out=ratio, in0=NM, in1=B2)
                        if not trivial_act:
                            ANM = lnp.tile([P, SC], f32, tag="gen3", bufs=3,
                                           name=f"anm_{sc}_{dt_}")
                            nc.scalar.activation(out=ANM, in_=NM, func=AF.Abs)
                            RL = lnp.tile([P, SC], f32, tag="gen4", bufs=3,
                                          name=f"rl_{sc}_{dt_}")
                            nc.scalar.activation(out=RL, in_=ANM, func=AF.Relu,
                                                 bias=abe_pd[:, dt_:dt_ + 1])
                            nc.vector.tensor_scalar_add(out=ANM, in0=ANM,
                                                        scalar1=EPS)
                            nc.vector.reciprocal(out=ANM, in_=ANM)
                            nc.vector.tensor_mul(out=RL, in0=RL, in1=ANM)
                            nc.vector.tensor_mul(out=ratio, in0=ratio, in1=RL)
                    for zt, lst, pname in ((zr_t[dt_], z2r, "z2r"),
                                           (zi_t[dt_], z2i, "z2i")):
                        if b2:
                            tmp = lnp.tile([P, SC], f32, tag="lnB", bufs=2,
                                           name=f"{pname}t_{sc}_{dt_}")
                            nc.vector.tensor_mul(out=tmp, in0=zt, in1=ratio)
                            z2t = pair_from(zv, [P, SC], "zv", zv_bufs,
                                            f"{pname}_{sc}_{dt_}", tmp)
                        else:
                            z2t = zv.tile([P, SC], mdt, tag="zv", bufs=zv_bufs,
                                          name=f"{pname}_{sc}_{dt_}")
                            nc.vector.tensor_mul(out=z2t, in0=zt, in1=ratio)
                        lst[sc][dt_] = z2t

        # ------------------------------------------------------------------
        # Phase B: q/k/v projections (+ augmented-V head vectors)
        # ------------------------------------------------------------------
        kr_t = [[None] * NSC for _ in range(DT)]
        ki_t = [[None] * NSC for _ in range(DT)]
        vr_t = [None] * ST
        vi_t = [None] * ST
        hv_t = [None] * ST

        with ExitStack() as phB:
            wp = phB.enter_context(tc.tile_pool(name="wqkv", bufs=1))
            stg = phB.enter_context(tc.tile_pool(name="stgB", bufs=6))
            ktp = phB.enter_context(tc.tile_pool(name="ktp", bufs=32))
            hvp = phB.enter_context(tc.tile_pool(name="hvp", bufs=16))
            ps_w = phB.enter_context(tc.tile_pool(name="psB_w", bufs=2, space="PSUM"))
            ps_mm = phB.enter_context(tc.tile_pool(name="psB_mm", bufs=6, space="PSUM"))

            # ---- q: spill transposed q to DRAM ----
            wqr, _ = load_weightT(wp, stg, ps_w, dram["Wq_r"], (1.0,), "qr", ld_bufs=8)
            wqi, _ = load_weightT(wp, stg, ps_w, dram["Wq_i"], (1.0, -1.0), "qi", ld_bufs=8)
            for sc in range(NSC):
                for dt_ in range(DT):
                    dsl = slice(dt_ * P, (dt_ + 1) * P)
                    for wA, zA, wB, zB, comp, pname in (
                            (wqr[1.0], z2r, wqi[-1.0], z2i, "r", "qrs"),
                            (wqr[1.0], z2i, wqi[1.0], z2r, "i", "qis")):
                        ps = ps_mm.tile([P, SC], f32, tag="mmB", bufs=6,
                                        name=f"{pname}_ps_{sc}_{dt_}")
                        emit_group(ps,
                                   [(psl(wA[ki], dsl), zA[sc][ki]) for ki in range(DT)]
                                   + [(psl(wB[ki], dsl), zB[sc][ki]) for ki in range(DT)])
                        st_t = pair_from(stg, [P, SC], "qstg", 4,
                                         f"{pname}_{sc}_{dt_}", ps)
                        if b2:
                            nc.sync.dma_start(
                                out=qsp[(comp, "h")][dsl, sc * SC:(sc + 1) * SC],
                                in_=st_t[0])
                            nc.sync.dma_start(
                                out=qsp[(comp, "l")][dsl, sc * SC:(sc + 1) * SC],
                                in_=st_t[1])
                        else:
                            nc.sync.dma_start(
                                out=qsp[(comp, "")][dsl, sc * SC:(sc + 1) * SC],
                                in_=st_t)

            # ---- k: keep transposed k resident ----
            wkr, _ = load_weightT(wp, stg, ps_w, dram["Wk_r"], (1.0,), "kr", ld_bufs=8)
            wki, _ = load_weightT(wp, stg, ps_w, dram["Wk_i"], (1.0, -1.0), "ki", ld_bufs=8)
            for sc in range(NSC):
                for dt_ in range(DT):
                    dsl = slice(dt_ * P, (dt_ + 1) * P)
                    for wA, zA, wB, zB, dst, pname in (
                            (wkr[1.0], z2r, wki[-1.0], z2i, kr_t, "krs"),
                            (wkr[1.0], z2i, wki[1.0], z2r, ki_t, "kis")):
                        ps = ps_mm.tile([P, SC], f32, tag="mmB", bufs=6,
                                        name=f"{pname}_ps_{sc}_{dt_}")
                        emit_group(ps,
                                   [(psl(wA[ki], dsl), zA[sc][ki]) for ki in range(DT)]
                                   + [(psl(wB[ki], dsl), zB[sc][ki]) for ki in range(DT)])
                        dst[dt_][sc] = pair_from(ktp, [P, SC], "kt", 32,
                                                 f"{pname}_{sc}_{dt_}", ps)

            # ---- v (natural layout) + head vectors ----
            wvr, nat_vr = load_weightT(wp, stg, ps_w, dram["Wv_r"], (1.0,), "vr", ld_bufs=8)
            wvi, nat_vi = load_weightT(wp, stg, ps_w, dram["Wv_i"], (1.0, -1.0), "vi", ld_bufs=8)

            # fh[, t, h] = filt * W_head_half   (r and i halves + negated r)
            fh_r = const.tile([P, DT, 3], f32, tag="fh_r", name="fh_r")
            fh_i = const.tile([P, DT, 3], f32, tag="fh_i", name="fh_i")
            for h, nm in enumerate(HEAD_W):
                nc.sync.dma_start(
                    out=fh_r[:, :, h:h + 1],
                    in_=dram[nm][:, 0:D].rearrange("o (t p) -> p t o", p=P))
                nc.sync.dma_start(
                    out=fh_i[:, :, h:h + 1],
                    in_=dram[nm][:, D:2 * D].rearrange("o (t p) -> p t o", p=P))
            for t in range(DT):
                nc.vector.tensor_scalar_mul(out=fh_r[:, t, :], in0=fh_r[:, t, :],
                                            scalar1=filt_pd[:, t:t + 1])
                nc.vector.tensor_scalar_mul(out=fh_i[:, t, :], in0=fh_i[:, t, :],
                                            scalar1=filt_pd[:, t:t + 1])
            fh_rn = const.tile([P, DT, 3], f32, tag="fh_rn", name="fh_rn")
            nc.scalar.mul(out=fh_rn, in_=fh_r, mul=-1.0)

            # u_r[di, h] = Wvr_nat.T @ fh_r + Wvi_nat.T @ fh_i
            # u_i[di, h] = Wvr_nat.T @ fh_i - Wvi_nat.T @ fh_r
            u_r = [None] * DT
            u_i = [None] * DT
            for ti in range(DT):
                tsl = slice(ti * P, (ti + 1) * P)
                for natA, rhsA, natB, rhsB, dst, pname in (
                        (nat_vr, fh_r, nat_vi, fh_i, u_r, "u_r"),
                        (nat_vr, fh_i, nat_vi, fh_rn, u_i, "u_i")):
                    ps = ps_w.tile([P, 3], f32, tag="ps_w", bufs=2,
                                   name=f"{pname}_ps_{ti}")
                    for to in range(DT):
                        nc.tensor.matmul(ps, natA[to][:, tsl],
                                         rhsA[:, to, :], start=(to == 0),
                                         stop=False)
                    for to in range(DT):
                        nc.tensor.matmul(ps, natB[to][:, tsl],
                                         rhsB[:, to, :], start=False,
                                         stop=(to == DT - 1))
                    dst[ti] = pair_from(const, [P, 3], f"{pname}{ti}", 1,
                                        f"{pname}_{ti}", ps)

            for sc in range(NSC):
                ps_v = []
                for stl in range(SC // P):
                    st = sc * (SC // P) + stl
                    sl = slice(stl * P, (stl + 1) * P)
                    for zA, wA, zB, wB, pname in (
                            (z2r, wvr[1.0], z2i, wvi[-1.0], "vrs"),
                            (z2i, wvr[1.0], z2r, wvi[1.0], "vis")):
                        ps = ps_mm.tile([P, D], f32, tag="mmB", bufs=6,
                                        name=f"{pname}_ps_{st}")
                        emit_group(ps,
                                   [(psl(zA[sc][ki], sl), wA[ki]) for ki in range(DT)]
                                   + [(psl(zB[sc][ki], sl), wB[ki]) for ki in range(DT)])
                        ps_v.append(ps)
                    psh = ps_w.tile([P, 3], f32, tag="ps_w", bufs=2,
                                    name=f"hv_ps_{st}")
                    if b2:
                        emit_group(psh,
                                   [(psl(z2r[sc][ki], sl), u_r[ki]) for ki in range(DT)]
                                   + [(psl(z2i[sc][ki], sl), u_i[ki]) for ki in range(DT)])
                    else:
                        for ki in range(DT):
                            nc.tensor.matmul(psh, z2r[sc][ki][:, sl].bitcast(f32),
                                             u_r[ki], start=(ki == 0), stop=False)
                        for ki in range(DT):
                            nc.tensor.matmul(psh, z2i[sc][ki][:, sl].bitcast(f32),
                                             u_i[ki], start=False,
                                             stop=(ki == DT - 1))
                    if b2:
                        hvh = hvp.tile([P, 4], bf16, tag="hvh", bufs=16,
                                       name=f"hvh_{st}")
                        nc.scalar.copy(out=hvh[:, 0:1], in_=ones_col)
                        nc.scalar.copy(out=hvh[:, 1:4], in_=psh)
                        hvl = hvp.tile([P, 4], bf16, tag="hvl", bufs=16,
                                       name=f"hvl_{st}")
                        nc.gpsimd.memset(hvl[:, 0:1], 0.0)
                        nc.vector.scalar_tensor_tensor(
                            out=hvl[:, 1:4], in0=hvh[:, 1:4], scalar=-1.0,
                            in1=psh, op0=ALU.mult, op1=ALU.add)
                        hv_t[st] = (hvh, hvl)
                    else:
                        hv = hvp.tile([P, 4], f32, tag="hv", bufs=16,
                                      name=f"hv_{st}")
                        nc.scalar.copy(out=hv[:, 0:1], in_=ones_col)
                        nc.scalar.copy(out=hv[:, 1:4], in_=psh)
                        hv_t[st] = hv
                # copybacks (v = psum * filt_bc); reuse z2 slots in zv pool
                for stl in range(SC // P):
                    st = sc * (SC // P) + stl
                    for j, (lst, pname) in enumerate(((vr_t, "vrf"),
                                                      (vi_t, "vif"))):
                        psv = ps_v[2 * stl + j]
                        if b2:
                            tmp = stg.tile([P, D], f32, tag="vtmp", bufs=2,
                                           name=f"{pname}t_{st}")
                            nc.vector.tensor_tensor(out=tmp, in0=psv,
                                                    in1=filt_bc, op=ALU.mult)
                            lst[st] = pair_from(zv, [P, D], "zv", zv_bufs,
                                                f"{pname}_{st}", tmp)
                        else:
                            v = zv.tile([P, D], mdt, tag="zv", bufs=zv_bufs,
                                        name=f"{pname}_{st}")
                            nc.vector.tensor_tensor(out=v, in0=psv,
                                                    in1=filt_bc, op=ALU.mult)
                            lst[st] = v

        # ------------------------------------------------------------------
        # Phase C: attention (scoresT -> exp -> attn@[v|heads] -> outputs)
        # ------------------------------------------------------------------
        with ExitStack() as phC:
            qp = phC.enter_context(tc.tile_pool(name="qp", bufs=16))
            ep = phC.enter_context(tc.tile_pool(name="ep", bufs=18))
            opool = phC.enter_context(tc.tile_pool(name="outp", bufs=3))
            hp = phC.enter_context(tc.tile_pool(name="headp", bufs=1))
            ps_s = phC.enter_context(tc.tile_pool(name="psC_s", bufs=2, space="PSUM"))
            ps_o = phC.enter_context(tc.tile_pool(name="psC_o", bufs=4, space="PSUM"))
            ps_h = phC.enter_context(tc.tile_pool(name="psC_h", bufs=2, space="PSUM"))

            heads_s = hp.tile([P, ST], f32, tag="hs", name="heads_s")
            heads_c = hp.tile([P, ST], f32, tag="hc", name="heads_c")
            heads_h = hp.tile([P, ST], f32, tag="hh", name="heads_h")
            scale = float(D ** -0.5)

            for qc in range(NQC):
                qr_c, qi_c = [], []
                for comp, lst in (("r", qr_c), ("i", qi_c)):
                    for dt_ in range(DT):
                        dsl = slice(dt_ * P, (dt_ + 1) * P)
                        csl = slice(qc * QC, (qc + 1) * QC)
                        if b2:
                            th = qp.tile([P, QC], bf16, tag="qTh", bufs=16,
                                         name=f"q{comp}h_c{qc}_{dt_}")
                            nc.sync.dma_start(out=th, in_=qsp[(comp, "h")][dsl, csl])
                            tl = qp.tile([P, QC], bf16, tag="qTl", bufs=16,
                                         name=f"q{comp}l_c{qc}_{dt_}")
                            nc.sync.dma_start(out=tl, in_=qsp[(comp, "l")][dsl, csl])
                            lst.append((th, tl))
                        else:
                            t = qp.tile([P, QC], mdt, tag="qT", bufs=16,
                                        name=f"q{comp}_c{qc}_{dt_}")
                            nc.sync.dma_start(out=t, in_=qsp[(comp, "")][dsl, csl])
                            lst.append(t)
                exps = []
                for kt in range(KT):
                    ps = ps_s.tile([P, QC], f32, tag="ps_s", bufs=2,
                                   name=f"sc_ps_{qc}_{kt}")
                    kq = SC // P
                    ksl = slice((kt % kq) * P, (kt % kq) * P + P)
                    ksc = kt // kq
                    emit_group(ps,
                               [(psl(kr_t[ki][ksc], ksl), qr_c[ki]) for ki in range(DT)]
                               + [(psl(ki_t[ki][ksc], ksl), qi_c[ki]) for ki in range(DT)])
                    if b2:
                        e32 = ep.tile([P, QC], f32, tag="e32", bufs=3,
                                      name=f"e32_{qc}_{kt}")
                        nc.scalar.activation(out=e32, in_=ps, func=AF.Exp,
                                             scale=scale)
                        e = pair_from(ep, [P, QC], "exp", 18,
                                      f"exp_{qc}_{kt}", e32)
                    else:
                        e = ep.tile([P, QC], mdt, tag="exp", bufs=18,
                                    name=f"exp_{qc}_{kt}")
                        nc.scalar.activation(out=e, in_=ps, func=AF.Exp,
                                             scale=scale)
                    exps.append(e)
                for qt in range(QC // P):
                    qti = qc * (QC // P) + qt
                    s0 = qti * P
                    pso_r = ps_o.tile([P, D], f32, tag="ps_o", bufs=4,
                                      name=f"or_ps_{qti}")
                    pso_i = ps_o.tile([P, D], f32, tag="ps_o", bufs=4,
                                      name=f"oi_ps_{qti}")
                    psh = ps_h.tile([P, 4], f32, tag="ps_h", bufs=2,
                                    name=f"h_ps_{qti}")
                    qsl = slice(qt * P, (qt + 1) * P)
                    emit_group(pso_r, [(psl(exps[kt], qsl), vr_t[kt])
                                       for kt in range(KT)])
                    emit_group(pso_i, [(psl(exps[kt], qsl), vi_t[kt])
                                       for kt in range(KT)])
                    if b2:
                        emit_group(psh, [(psl(exps[kt], qsl), hv_t[kt])
                                         for kt in range(KT)])
                    else:
                        for kt in range(KT):
                            nc.tensor.matmul(psh, exps[kt][:, qsl].bitcast(f32),
                                             hv_t[kt], start=(kt == 0),
                                             stop=(kt == KT - 1))
                    rc = opool.tile([P, 1], f32, tag="recip", bufs=4,
                                    name=f"rc_{qti}")
                    nc.vector.reciprocal(out=rc, in_=psh[:, 0:1])
                    prt = opool.tile([P, D], f32, tag="pr", bufs=3,
                                     name=f"prt_{qti}")
                    nc.vector.tensor_scalar_mul(out=prt, in0=pso_r, scalar1=rc)
                    nc.sync.dma_start(out=out_pr[s0:s0 + P, :], in_=prt)
                    pit = opool.tile([P, D], f32, tag="pr", bufs=3,
                                     name=f"pit_{qti}")
                    nc.vector.tensor_scalar_mul(out=pit, in0=pso_i, scalar1=rc)
                    nc.sync.dma_start(out=out_pi[s0:s0 + P, :], in_=pit)
                    nc.vector.tensor_scalar(out=heads_s[:, qti:qti + 1],
                                            in0=psh[:, 1:2], scalar1=rc,
                                            scalar2=b_score_bc, op0=ALU.mult,
                                            op1=ALU.add)
                    nc.scalar.activation(out=heads_c[:, qti:qti + 1],
                                         in_=psh[:, 2:3], func=AF.Sigmoid,
                                         scale=rc, bias=b_conf_bc)
                    nc.vector.tensor_scalar(out=heads_h[:, qti:qti + 1],
                                            in0=psh[:, 3:4], scalar1=rc,
                                            scalar2=b_halt_bc, op0=ALU.mult,
                                            op1=ALU.add)

            # heads: [P, ST] -> transpose -> [ST, P] -> contiguous DMA
            for src, dst in ((heads_s, out_sc), (heads_c, out_cf),
                             (heads_h, out_hl)):
                pst = ps_h.tile([ST, P], f32, tag="ps_h", bufs=2,
                                name=f"hT_ps_{src.name}")
                nc.tensor.matmul(pst, src, ident, is_transpose=True,
                                 start=True, stop=True, skip_group_check=True)
                sb = opool.tile([ST, P], f32, tag="hT", bufs=2,
                                name=f"hT_{src.name}")
                nc.scalar.copy(out=sb, in_=pst)
                nc.sync.dma_start(
                    out=dst.rearrange("(t p) o -> t (p o)", p=P), in_=sb)

    nc.compile()
    return nc


_CACHE = {}


def _get_program(S, mm_mode, trivial_ln, trivial_act):
    key = (S, mm_mode, trivial_ln, trivial_act)
    if key not in _CACHE:
        _CACHE[key] = build_program(S, mm_mode, trivial_ln, trivial_act)
    return _CACHE[key]


def kernel(**inputs):
    inputs = {k: np.asarray(v, dtype=np.float32) if np.asarray(v).dtype != np.float32
              else np.asarray(v) for k, v in inputs.items()}
    B, S, D_ = inputs["raw_real"].shape
    assert B == N_CORES and D_ == D
    trivial_ln = bool(np.all(inputs["ln_scale"] == 1.0)
                      and np.all(inputs["ln_shift"] == 0.0))
    trivial_act = bool(np.all(inputs["act_bias"] == 0.0))
    mm_mode = os.environ.get("MM_MODE", "b2")
    nc = _get_program(S, mm_mode, trivial_ln, trivial_act)

    shared = {}
    for nm in W_NAMES + VEC_NAMES + HEAD_W + HEAD_B:
        shared[nm] = np.ascontiguousarray(inputs[nm])
    in_maps = []
    for b in range(B):
        m = dict(shared)
        for nm in ("raw_real", "raw_imag", "bs_real", "bs_imag"):
            m[nm] = np.ascontiguousarray(inputs[nm][b])
        in_maps.append(m)

    res = run_bass_kernel_spmd(nc, in_maps, core_ids=list(range(N_CORES)))
    r = res.results
    pr = np.stack([r[b]["pr"] for b in range(B)])
    pi = np.stack([r[b]["pi"] for b in range(B)])
    score = np.stack([r[b]["score"] for b in range(B)])
    conf = np.stack([r[b]["confidence"] for b in range(B)])
    halt = np.stack([r[b]["halt"] for b in range(B)])
    filt = r[0]["filt"]
    return (pr, pi, score, conf, halt, filt)


if __name__ == "__main__":
    nc = build_program(S=512)
    print("built ok")
